# revision 1
# baseline (speedup 1.0000x reference)
"""Trainium2 Bass kernel for nn_Model_11888469475981 (pooling).

Reference semantics (per sample n, channel c):
  x_p = zeropad3d(x, W:(1,2), H:(1,1), D:(0,1))           # (17, 66, 259)
  rows = x_p rows along W (each length Wp=259), K=3 S=2 maxpool w/ indices,
  softsign, max-unpool scatter, add x_p, mean over padded D (17).

Key restructure (exact, no gather/scatter):
  For a padded row A[0..258], position w receives softsign(A[w]) iff some
  window picks w as its (first-occurrence) argmax. With L[w] = [A[w] > A[w-1]]
  and e1[m] = [A[2m] >= A[2m+2]]:
    odd w=2m+1 : mask = L[2m+1] * (1 - L[2m+2])
    even w=2m  : mask = max( (1-L[2m+1])*e1[m],  (1-e1[m-1])*L[2m] )
  fused[w] = A[w] * (1 + mask[w] * 1/(1+|A[w]|))
  out[h, w] = (1/17) * sum_d fused[d, h, w]   (padded D slab and padded H/W
  rows are exactly zero and are written as zeros / via the 1/17 weight).

Layout per core (1 sample): per channel c, one SBUF tile [128, 8*264]:
  partition p = d*8 + hg  (d in 0..15, hg = h//8), free = hs-slot (h%8) * 264.
  Slot: [2 guard][259 padded-W cols][3 guard], real x at cols 3..258.
  Depth-mean via PE matmul with lhsT W8[p, m] = (1/17)*[p%8 == m] -> psum[8,264].
"""

import numpy as np

import concourse.bass as bass
import concourse.mybir as mybir
from concourse import bacc
from concourse.tile import TileContext
from concourse.bass_utils import run_bass_kernel_spmd

N_CORES = 8
C, D, H, W = 32, 16, 64, 256
HP, WP = 66, 259
SLOT = 264
NS = 8              # h-subslots per partition
FREE = NS * SLOT
DSLOT = 132         # dense (per-window-index m) slot width
DFREE = NS * DSLOT
F32 = mybir.dt.float32
Alu = mybir.AluOpType
Act = mybir.ActivationFunctionType


def _fullw(t, c0, cnt):
    return t[:].rearrange("p (s w) -> p s w", s=NS)[:, :, c0:c0 + cnt]


def _dense(t, c0, cnt):
    return t[:].rearrange("p (s w) -> p s w", s=NS)[:, :, c0:c0 + cnt]


def _v2(t):
    return t[:].rearrange("p (s w2 two) -> p s w2 two", s=NS, two=2)


def _ev(t, mshift, cnt):
    # even padded-w columns: col = 2 + 2*(m + mshift), m in [0, cnt)
    return _v2(t)[:, :, 1 + mshift:1 + mshift + cnt, 0]


def _od(t, mshift, cnt):
    # odd padded-w columns: col = 3 + 2*(m + mshift), m in [0, cnt)
    return _v2(t)[:, :, 1 + mshift:1 + mshift + cnt, 1]


def build_nc():
    # Bacc: its finalize() runs the wait-splitting / legalization passes
    # (TRN2 allows at most 1 sync wait per instruction).
    nc = bacc.Bacc()
    x_ext = nc.declare_dram_parameter("x", [C, D, H, W], F32, isOutput=False)
    w8_ext = nc.declare_dram_parameter("w8", [128, 8], F32, isOutput=False)
    out_ext = nc.declare_dram_parameter("out", [C, HP, WP], F32, isOutput=True)

    with TileContext(nc) as tc:
        with tc.tile_pool(name="main", bufs=1) as pool, \
             tc.tile_pool(name="psum", bufs=2, space="PSUM") as psum_pool:
            a_ts = [pool.tile([128, FREE], F32, tag=f"a{i}", name=f"a{i}") for i in range(3)]
            f_ts = [pool.tile([128, FREE], F32, tag=f"fu{i}", name=f"fu{i}") for i in range(2)]
            m2_ts = [pool.tile([128, FREE], F32, tag=f"m2{i}", name=f"m2{i}") for i in range(2)]
            r_ts = [pool.tile([128, FREE], F32, tag=f"r{i}", name=f"r{i}") for i in range(2)]
            ab_t = pool.tile([128, FREE], F32, tag="abs", name="abs")
            ln_t = pool.tile([128, FREE], F32, tag="lnt", name="lnt")
            l_t = pool.tile([128, FREE], F32, tag="lcmp", name="lcmp")
            e1_t = pool.tile([128, DFREE], F32, tag="e1", name="e1")
            to_t = pool.tile([128, DFREE], F32, tag="todd", name="todd")
            fe_t = pool.tile([128, DFREE], F32, tag="fev", name="fev")
            le_t = pool.tile([128, DFREE], F32, tag="lev", name="lev")
            mk_t = pool.tile([128, DFREE], F32, tag="mask", name="mask")
            w8_t = pool.tile([128, 8], F32, tag="w8", name="w8")
            o_ts = [pool.tile([8, NS * WP], F32, tag=f"o{i}", name=f"o{i}")
                    for i in range(2)]
            z_t = pool.tile([32, 2 * WP], F32, tag="zrow", name="zrow")

            # one-time init: zero guards (and any never-written-but-read cols).
            # Memsets run on DVE so downstream DVE/PE consumers do not need an
            # extra cross-engine semaphore wait (walrus caps waits per inst).
            for t in a_ts + f_ts + m2_ts + r_ts:
                nc.vector.memset(t[:], 0.0)
            nc.vector.memset(l_t[:], 0.0)
            nc.vector.memset(e1_t[:], 0.0)
            nc.gpsimd.memset(z_t[:], 0.0)
            nc.sync.dma_start(out=w8_t[:], in_=w8_ext[:, :])

            # padded-H border rows (h'=0 and h'=65) for every channel: zeros
            nc.sync.dma_start(
                out=bass.AP(out_ext, 0, [[HP * WP, C], [65 * WP, 2], [1, WP]]),
                in_=z_t[:].rearrange("p (a w) -> p a w", w=WP),
            )

            for c in range(C):
                a_t = a_ts[c % 3]
                F_t = f_ts[c % 2]
                m2_t = m2_ts[c % 2]
                r_t = r_ts[c % 2]

                # load channel: rows r=(d*64+h) -> partition p=d*8+h//8, slot h%8
                av = a_t[:].rearrange("p (s w) -> p s w", s=NS)
                nc.sync.dma_start(
                    out=av[:, :, 3:259],
                    in_=bass.AP(
                        x_ext,
                        c * D * H * W,
                        [[2048, 128], [256, NS], [1, W]],
                    ),
                )

                # L[w] = A[w] > A[w-1], w=0..258 (cols 2..260)
                nc.vector.tensor_tensor(
                    _fullw(l_t, 2, 259), _fullw(a_t, 2, 259), _fullw(a_t, 1, 259),
                    Alu.is_gt)
                # e1[m] = A[2m] >= A[2m+2], m=0..129
                nc.vector.tensor_tensor(
                    _dense(e1_t, 2, 130), _ev(a_t, 0, 130), _ev(a_t, 1, 130),
                    Alu.is_ge)
                # odd mask: todd[m] = (L[2m+2]==0) * L[2m+1], m=0..128
                nc.vector.scalar_tensor_tensor(
                    _dense(to_t, 2, 129), _ev(l_t, 1, 129), 0.0, _od(l_t, 0, 129),
                    Alu.is_equal, Alu.mult)
                # even "first": fe[m] = (L[2m+1]==0) * e1[m], m=0..129
                nc.vector.scalar_tensor_tensor(
                    _dense(fe_t, 2, 130), _od(l_t, 0, 130), 0.0, _dense(e1_t, 2, 130),
                    Alu.is_equal, Alu.mult)
                # even "last": le[m] = (e1[m-1]==0) * L[2m], m=0..129
                nc.vector.scalar_tensor_tensor(
                    _dense(le_t, 2, 130), _dense(e1_t, 1, 130), 0.0, _ev(l_t, 0, 130),
                    Alu.is_equal, Alu.mult)
                # even mask = max(first, last)
                nc.vector.tensor_tensor(
                    _dense(mk_t, 2, 130), _dense(fe_t, 2, 130), _dense(le_t, 2, 130),
                    Alu.max)

                # softsign reciprocal on ACT: r = 1/(1+|A|) = sigmoid(-ln|A|).
                # Only real cols 3..258; r at pad cols stays 0 from the
                # one-time memset (m2 = mask*0 = 0 there, and A=0 -> F=0).
                nc.scalar.activation(_fullw(ab_t, 3, 256), _fullw(a_t, 3, 256),
                                     Act.Abs)
                nc.scalar.activation(_fullw(ln_t, 3, 256), _fullw(ab_t, 3, 256),
                                     Act.Ln)
                nc.scalar.activation(_fullw(r_t, 3, 256), _fullw(ln_t, 3, 256),
                                     Act.Sigmoid, scale=-1.0)

                # m2 = mask * r  (parity-split writes)
                nc.vector.tensor_tensor(
                    _od(m2_t, 0, 129), _dense(to_t, 2, 129), _od(r_t, 0, 129),
                    Alu.mult)
                nc.vector.tensor_tensor(
                    _ev(m2_t, 0, 130), _dense(mk_t, 2, 130), _ev(r_t, 0, 130),
                    Alu.mult)
                # fused = (m2 + 1) * A
                nc.vector.scalar_tensor_tensor(
                    _fullw(F_t, 2, 260), _fullw(m2_t, 2, 260), 1.0,
                    _fullw(a_t, 2, 260), Alu.add, Alu.mult)

                # depth-sum via PE: psum[hg, w] = sum_d F[(d,hg), w], then
                # ScalarE evacuates PSUM->SBUF applying the 1/17 mean scale.
                Fv = F_t[:].rearrange("p (s w) -> p s w", s=NS)
                osb = o_ts[c % 2]
                ov = osb[:].rearrange("p (s w) -> p s w", s=NS)
                for half in range(2):
                    ps = psum_pool.tile([8, 4 * 512], F32, tag="ps",
                                        name=f"ps_{c}_{half}")
                    psv = ps[:].rearrange("p (s w) -> p s w", s=4)
                    for k in range(4):
                        hs = half * 4 + k
                        nc.tensor.matmul(psv[:, k, 0:SLOT], w8_t[:, 0:8],
                                         Fv[:, hs, :], start=True, stop=True)
                    nc.scalar.mul(ov[:, 4 * half:4 * half + 4, :],
                                  psv[:, :, 2:261], 1.0 / 17.0)
                nc.sync.dma_start(
                    out=bass.AP(out_ext, (c * HP + 1) * WP,
                                [[8 * WP, 8], [WP, NS], [1, WP]]),
                    in_=ov[:, :, :],
                )
    nc.finalize()
    return nc


_CACHE: dict = {}


def _get_nc():
    if "nc" not in _CACHE:
        _CACHE["nc"] = build_nc()
    return _CACHE["nc"]


def make_in_maps(x: np.ndarray):
    w8 = np.zeros((128, 8), np.float32)
    w8[np.arange(128), np.arange(128) % 8] = 1.0
    return [
        {"x": np.ascontiguousarray(x[i]), "w8": w8}
        for i in range(N_CORES)
    ]


def kernel(**inputs) -> np.ndarray:
    x = np.ascontiguousarray(np.asarray(inputs["x"], dtype=np.float32))
    assert x.shape == (N_CORES, C, D, H, W), x.shape
    nc = _get_nc()
    res = run_bass_kernel_spmd(nc, make_in_maps(x), list(range(N_CORES)))
    return np.stack([res.results[i]["out"] for i in range(N_CORES)], axis=0)

